# revision 42
# baseline (speedup 1.0000x reference)
"""Backward_projection (FBP: ramp filter + backprojection) on 8 trn2 NeuronCores.

Device formulation (hand-written Bass/Tile kernel, one NEFF per core, SPMD):
  * The ramp filter (exact 183x183 circulant-section matmul, all scalar
    factors folded) is FOLDED into the backprojection matrix:
    out[b, p] = sum_d x[b, a, d] * Wt_a[d, p],  Wt_a = F @ tri(d - k[a, p]),
    so the whole FBP is one accumulation of per-angle matmuls on the PE.
  * Angle-flip pairing: k_{284-a}(127-i, j) == k_a(i, j) exactly, so the
    weight tile of (angle a, image row i) also serves (284-a, 127-i); each
    matmul streams N=512 (256 batches x 2 pair-halves) through one
    stationary tile.  Angle 142 (theta = pi/2) pairs with itself: its
    flip-half writes the mirrored row, so it rides the same uniform loop.
  * K = 183 splits into chunks of 128 + 55; the 55-chunks of two adjacent
    pairs are packed into one [110, .] stationary tile (25% fewer streams).
  * Sharding: each core owns the symmetric image-row set
    [8c, 8c+8) u [120-8c, 128-8c) (16 rows, 2048 pixels) for all 256
    batches, accumulated over two 8-row PSUM sweeps.  The sinogram wire is
    PAIR-sharded pair-major (18 pair-slots per core, both flip angles
    adjacent), so the in-kernel AllGather output is consumed directly with
    no DRAM rearrange.
  * Weights (~107MB/core bf16) are GENERATED ON DEVICE once from a small
    geometry table, stored sweep-major, and streamed per call in ~1MB
    chunks (4 pairs per DMA) for near-peak HBM bandwidth.
  * Output is written pixel-major ([16 rows, 128 cols, 256 batches] per
    core) straight from PSUM via DVE copies -- no on-chip transposes; the
    host reassembles (cheap) while fusing the dequant scale.

The wire uses int8 (validated against the reference: quantization noise
contributes ~1.4e-2 of the 2e-2 absmax-relative budget; the bf16 path at
~5e-3 is kept as an automatic fallback for inputs exceeding the int8
range).  Falls back to a host numba/numpy implementation if the device
path is unavailable.
"""

import numpy as np

# --- geometry constants (parallel_beam_geometry on a 128^2 grid) ---
N = 128
CELL = 40.0 / N
RHO = float(np.sqrt(2.0) * 20.0)
A = 285
D = 183
DC = 2.0 * RHO / D
PAD = 512
B = 256
NPAIR = 143          # pairs (t, 284-t) for t<142, plus the self-pair (142,142)
NSLOT = 144          # wire pair-slots (143 + 1 zero pad) = 18 per core
INT8_SCALE = 127.0 / 5.5
INT8_ABSMAX = 5.45  # inputs beyond this use the bf16 wire


def _filter_matrix():
    n = (np.fft.fftfreq(PAD) * PAD).astype(np.int64)
    h = np.zeros(PAD, np.float64)
    h[0] = 1.0 / (4.0 * DC * DC)
    odd = (n % 2) != 0
    h[odd] = -1.0 / (np.pi * n[odd] * DC) ** 2
    idx = (np.arange(D)[None, :] - np.arange(D)[:, None]) % PAD
    return (h[idx] * (12.0 * DC * np.pi / A)).astype(np.float32)


def _k_eff():
    c = -20.0 + (np.arange(N) + 0.5) * CELL
    X, Y = np.meshgrid(c, c, indexing="ij")
    th = (np.arange(A) + 0.5) * np.pi / A
    t = np.cos(th)[:, None] * X.ravel()[None, :] + np.sin(th)[:, None] * Y.ravel()[None, :]
    k = (t - (-RHO + 0.5 * DC)) / DC
    k0 = np.clip(np.floor(k), 0, D - 2)
    w = np.clip(k - k0, 0.0, 1.0)
    return (k0 + w).astype(np.float32).reshape(A, N, N)


_DEV = None
_DEV_FAILED = False


def _init_device():
    global _DEV, _DEV_FAILED
    if _DEV is not None:
        return _DEV
    if _DEV_FAILED:
        raise RuntimeError("device init previously failed")

    import jax
    import jax.numpy as jnp
    import ml_dtypes
    from functools import partial
    import concourse.bass as bass  # noqa: F401  (ensures concourse importable)
    import concourse.mybir as mybir
    from concourse.tile import TileContext
    from concourse.bass2jax import bass_jit, bass_shard_map
    from jax.sharding import Mesh, NamedSharding, PartitionSpec as P
    from jax.experimental.shard_map import shard_map

    BF = mybir.dt.bfloat16
    F32d = mybir.dt.float32
    I8 = mybir.dt.int8

    F = _filter_matrix()
    KE = _k_eff()
    # kg[c, t, i, j]: fractional detector index for pair t at this core's
    # row set (rows [8c,8c+8) u [120-8c,128-8c)), 143 pairs incl. (142,142)
    kg_host = np.stack([
        np.concatenate([KE[0:NPAIR, 8 * c:8 * c + 8, :],
                        KE[0:NPAIR, 120 - 8 * c:128 - 8 * c, :]], axis=1)
        for c in range(8)])                    # [8, 143, 16, 128]

    devs = jax.devices()[:8]
    assert len(devs) == 8
    mesh = Mesh(np.array(devs), ("core",))
    shard0 = NamedSharding(mesh, P("core"))

    NG = (NPAIR + 1) // 2  # 72 hi-det pair-groups (last one half-empty)
    GCH = 3                # AllGather chunks; 18/GCH slots per core per chunk
    SLOC = NSLOT // 8      # 18 pair-slots per core
    CSL = SLOC // GCH      # 6 slots per core per chunk
    # processing order: chunk-major so compute on gathered chunk g overlaps
    # the still-in-flight gather of chunk g+1 (slot 143 is the zero pad and
    # lands last)
    PERM = np.array([18 * c + l for g in range(GCH) for c in range(8)
                     for l in range(CSL * g, CSL * (g + 1))], np.int64)
    NPROC = NSLOT  # 144 processed slots (incl. the zero-padded one)

    @partial(jax.jit, out_shardings=(shard0,) * 2)
    def gen_w(kg, Fd):
        def body(kl, Fd):
            j = jnp.arange(D, dtype=jnp.float32)
            klf = kl.reshape(NPAIR, 16 * 128)
            Wblk = jax.nn.relu(1.0 - jnp.abs(j[None, :, None] - klf[:, None, :]))
            Wt = jnp.einsum("dj,tjq->tdq", Fd, Wblk)        # [143,183,2048]
            # pair (142,142) delivers angle 142 twice to every row (once via
            # flip0, once as the mirror row's flip1) -> halve its weights
            Wt = Wt.at[NPAIR - 1].mul(0.5)
            Wt = Wt.reshape(NPAIR, D, 16, 128)
            # pad the unused slot 143 with zero weights, order by PERM
            Wt = jnp.concatenate(
                [Wt, jnp.zeros((1, D, 16, 128), Wt.dtype)], axis=0)[PERM]
            # DMA-ready sweep-major low-det weights: [2, 36, 128, 4, 8, 128]
            # (chunk T = processing slots 4T..4T+3)
            w0p = Wt[:, 0:128].reshape(36, 4, 128, 16, 128).transpose(0, 2, 1, 3, 4)
            w0 = jnp.stack([w0p[..., 0:8, :], w0p[..., 8:16, :]],
                           axis=0).astype(jnp.bfloat16)
            # packed hi-det weights: [2, 36, 110, 2, 8, 128] (group g =
            # processing slots 2g, 2g+1; chunk = 2 groups)
            w1s = Wt[:, 128:183, :, :]                       # [144, 55, 16, 128]
            w1s = w1s.reshape(36, 2, 110, 16, 128).transpose(0, 2, 1, 3, 4)
            w1 = jnp.stack([w1s[..., 0:8, :], w1s[..., 8:16, :]],
                           axis=0).astype(jnp.bfloat16)
            return w0, w1
        return shard_map(body, mesh=mesh, in_specs=(P("core"), P()),
                         out_specs=(P("core"),) * 2, check_rep=False)(kg, Fd)

    def make_fbp(R, ag="shared", nbufs=3, ribufs=6, dpbufs=2, diag=None,
                 trace_sim=False):
        # R = number of back-to-back repetitions of the full per-call
        # pipeline inside one NEFF.  R=1 is the production kernel; R>1
        # variants exist only so the marginal per-repetition device time can
        # be measured (the axon relay adds ~70ms of fixed per-dispatch RPC
        # overhead that would otherwise swamp the sub-ms kernel).
        # ag: "shared"|"internal" AllGather buffer kind, "off" = timing
        # ablation only (wrong data).

        @bass_jit(num_devices=8)
        def fbp_bass(nc, xs, w0, w1):
            # xs [18, 183, 2, 256]: this core's pair-slots, det-major
            # w0 [2, 36, 128, 4, 8, 128], w1 [2, 36, 110, 2, 8, 128]
            wire_dt = I8 if xs.dtype == I8 else BF
            out = nc.dram_tensor("out", [128, 16, 256], BF, kind="ExternalOutput")
            with TileContext(nc, trace_sim=trace_sim) as tc:
                with tc.tile_pool(name="wp", bufs=nbufs) as wp, \
                     tc.tile_pool(name="rp", bufs=ribufs) as rp, \
                     tc.tile_pool(name="ri", bufs=ribufs) as ri, \
                     tc.tile_pool(name="ps", bufs=8, space="PSUM") as ps, \
                     tc.tile_pool(name="sp", bufs=dpbufs) as sp, \
                     tc.tile_pool(name="dp", bufs=dpbufs, space="DRAM") as dp:
                 for _rep in range(R):
                    bounce_in = dp.tile([SLOC, D, 2, 256], wire_dt, tag="bin")
                    nc.sync.dma_start(out=bounce_in, in_=xs[:, :, :, :])
                    if ag == "one":
                        # single collective: lowest entry/exit overhead; the
                        # dpbufs=2 double buffering hides it under the
                        # previous repetition's compute in steady state
                        bout1 = dp.tile([8, SLOC, D, 2, 256], wire_dt,
                                        tag="bout1", name="bout1",
                                        addr_space="Shared")
                        nc.gpsimd.collective_compute(
                            "AllGather", mybir.AluOpType.bypass,
                            replica_groups=[list(range(8))],
                            ins=[bounce_in], outs=[bout1])
                        bounce_out = [bout1]
                    else:
                        bounce_out = [
                            dp.tile([8, CSL, D, 2, 256], wire_dt, tag=f"bout{g}",
                                    name=f"bout{g}",
                                    addr_space="Shared" if ag == "shared" else "Local")
                            for g in range(GCH)]
                        if ag != "off":
                            for g in range(GCH):
                                nc.gpsimd.collective_compute(
                                    "AllGather", mybir.AluOpType.bypass,
                                    replica_groups=[list(range(8))],
                                    ins=[bounce_in[CSL * g:CSL * (g + 1)]],
                                    outs=[bounce_out[g]])

                    def slot(t):  # processing index -> location in bounce_out
                        s = int(PERM[t])
                        c, l = s // SLOC, s % SLOC
                        return (l // CSL, c, l % CSL)

                    def bsrc(gc, c, l):  # bounce slice for (chunk, core, local)
                        if ag == "one":
                            return bounce_out[0][c, gc * CSL + l]
                        return bounce_out[gc][c, l]

                    stg = sp.tile([128, 16, 256], BF, tag="stg")

                    # input loads ride the ACT HWDGE ring so they never queue
                    # FIFO behind the 1MB weight chunks on the SP ring
                    in_dma = nc.sync if diag == "syncin" else nc.scalar
                    phased = diag in ("phased", "ringph")
                    if diag == "rhsfix":  # timing ablation: resident rhs
                        r0fix = sp.tile([128, 2, 256], BF, tag="r0fix",
                                        name=f"r0fix_{_rep}")
                        nc.vector.memset(r0fix, 0.25)
                        r1fix = sp.tile([110, 2, 256], BF, tag="r1fix",
                                        name=f"r1fix_{_rep}")
                        nc.vector.memset(r1fix, 0.25)
                    # SBUF-resident int8 sinogram: each slot's data is DMA'd
                    # once (first use, sweep 0); sweep 1 re-reads SBUF with no
                    # input DMAs, removing PE stalls behind the weight stream
                    xlo = sp.tile([128, NPROC, 2, 256], wire_dt, tag="xlo",
                                  name=f"xlo_{_rep}", bufs=1)
                    xhi = sp.tile([110, NPROC // 2, 2, 256], wire_dt, tag="xhi",
                                  name=f"xhi_{_rep}", bufs=1)

                    def do_w1(sw, g, accs, last):
                        nonlocal w1c
                        if g % 2 == 0:
                            w1c = wp.tile([110, 2, 8, 128], BF, tag="w1",
                                          name=f"w1_{sw}_{g}")
                            nc.sync.dma_start(out=w1c, in_=w1[sw, g // 2])
                        if diag == "rhsfix":
                            r1 = r1fix
                        else:
                            if sw == 0:
                                for kk in range(2):
                                    gca, ca, la = slot(2 * g + kk)
                                    in_dma.dma_start(
                                        out=xhi[55 * kk:55 * kk + 55, g],
                                        in_=bsrc(gca, ca, la)[128:183])
                            if wire_dt == BF:
                                r1 = xhi[:, g]
                            else:
                                r1 = rp.tile([110, 2, 256], BF, tag="r1",
                                             name=f"r1_{sw}_{g}")
                                nc.vector.tensor_copy(r1, xhi[:, g])
                        for r in range(8):
                            nc.tensor.matmul(accs[r], lhsT=w1c[:, g % 2, r, :],
                                             rhs=r1, start=False, stop=last)

                    for sw in range(2):
                        accs = [ps.tile([128, 512], F32d, tag="acc",
                                        name=f"acc{sw}_{r}") for r in range(8)]
                        w0c = w1c = None
                        for t in range(NPROC):
                            if t % 4 == 0:
                                w0c = wp.tile([128, 4, 8, 128], BF, tag="w0",
                                              name=f"w0_{sw}_{t}")
                                nc.sync.dma_start(out=w0c, in_=w0[sw, t // 4])
                                if diag == "wx2":
                                    nc.sync.dma_start(out=w0c, in_=w0[sw, t // 4])
                            if diag == "rhsfix":
                                r0 = r0fix
                            else:
                                if sw == 0:
                                    gc, c, l = slot(t)
                                    in_dma.dma_start(out=xlo[:, t],
                                                     in_=bsrc(gc, c, l)[0:128])
                                if wire_dt == BF:
                                    r0 = xlo[:, t]
                                else:
                                    r0 = rp.tile([128, 2, 256], BF, tag="r0",
                                                 name=f"r0_{sw}_{t}")
                                    nc.vector.tensor_copy(r0, xlo[:, t])
                            for r in range(8):
                                nc.tensor.matmul(accs[r], lhsT=w0c[:, t % 4, r, :],
                                                 rhs=r0, start=(t == 0), stop=False)
                            if not phased and t % 2 == 1:
                                do_w1(sw, t // 2, accs, last=(t == NPROC - 1))
                        if phased:
                            for g in range(NPROC // 2):
                                do_w1(sw, g, accs, last=(g == NPROC // 2 - 1))

                        for r in range(8):
                            # flip0 -> slot (8*sw + r); flip1 -> mirrored slot
                            s0 = 8 * sw + r
                            s1 = (15 - r) if sw == 0 else (7 - r)
                            if sw == 0:
                                nc.vector.tensor_copy(stg[:, s0, :], accs[r][:, 0:256])
                                nc.vector.tensor_copy(stg[:, s1, :], accs[r][:, 256:512])
                            else:
                                nc.vector.tensor_add(stg[:, s0, :], stg[:, s0, :],
                                                     accs[r][:, 0:256])
                                nc.vector.tensor_add(stg[:, s1, :], stg[:, s1, :],
                                                     accs[r][:, 256:512])
                    nc.sync.dma_start(out=out[:, :, :], in_=stg[:, :, :])
            return out

        return bass_shard_map(fbp_bass, mesh=mesh,
                              in_specs=(P("core"),) * 3, out_specs=P("core"))

    fbp_dev = make_fbp(1)

    try:
        W0g, W1g = gen_w(kg_host.reshape(8 * NPAIR, 16, 128), F)
        for t_ in (W0g, W1g):
            t_.block_until_ready()
        # compile the int8-wire NEFF now so the first real call is fast
        dummy = np.zeros((NSLOT, D, 2, 256), np.int8)
        o0 = fbp_dev(dummy, W0g, W1g)
        o0.block_until_ready()
    except Exception:
        _DEV_FAILED = True
        raise

    _DEV = dict(jax=jax, np_bf16=np.dtype(ml_dtypes.bfloat16),
                fbp=fbp_dev, W=(W0g, W1g), in_sharding=shard0,
                make_fbp=make_fbp)
    return _DEV


_ROWSETS = [np.r_[8 * c:8 * c + 8, 120 - 8 * c:128 - 8 * c] for c in range(8)]


def _assemble(on, scale):
    # on: [1024, 16, 256] bf16-ish from device = [8 cores x 128 cols,
    # 16 row-slots, 256 batches]; reassemble to [256, 128, 128] f32 with
    # the dequant scale fused into the cast
    on = np.asarray(on).reshape(8, N, 16, B)
    img = np.empty((B, N, N), np.float32)
    for c in range(8):
        blk = np.multiply(on[c].transpose(2, 1, 0), scale, dtype=np.float32)
        img[:, _ROWSETS[c], :] = blk
    return img


_PACK_BUF = None


def _pack_wire(xq):
    # [256,285,183] -> pair-major pair-sharded wire [144, 183, 2, 256]:
    # slot t < 142: (angle t, angle 284-t); slot 142: (142, 142); slot 143: 0
    global _PACK_BUF
    if _PACK_BUF is None or _PACK_BUF.dtype != xq.dtype:
        _PACK_BUF = np.zeros((NSLOT, D, 2, B), xq.dtype)
    w = _PACK_BUF
    # xq[b, a, d] -> w[t, d, 0, b] = xq[b, t, d]; w[t, d, 1, b] = xq[b, 284-t, d]
    xt = xq.transpose(1, 2, 0)                       # [285, 183, 256]
    w[0:NPAIR, :, 0, :] = xt[0:NPAIR]
    w[0:142, :, 1, :] = xt[284:142:-1]
    w[142, :, 1, :] = xt[142]
    return w


_QBUFS = None
_QUANT_NUMBA = None


def _get_quant_numba():
    # single-pass fused quantizer: int8(rint(x*s)); ~2x the numpy 3-pass chain
    global _QUANT_NUMBA
    if _QUANT_NUMBA is None:
        try:
            import numba

            @numba.njit(cache=True)
            def q_(xf, s, out):
                for i in range(xf.size):
                    out[i] = np.int8(np.rint(xf[i] * s))

            q_(np.zeros(4, np.float32), 1.0, np.empty(4, np.int8))  # warm JIT
            _QUANT_NUMBA = q_
        except Exception:
            _QUANT_NUMBA = False
    return _QUANT_NUMBA


def _quant_pack_int8(x):
    global _QBUFS
    if _QBUFS is None:
        _QBUFS = (np.empty(x.shape, np.float32), np.empty(x.shape, np.int8))
    tmp, qi = _QBUFS
    qn = _get_quant_numba()
    if qn:
        qn(x.reshape(-1), INT8_SCALE, qi.reshape(-1))
    else:
        np.multiply(x, INT8_SCALE, out=tmp)
        np.rint(tmp, out=tmp)
        np.copyto(qi, tmp, casting="unsafe")
    return _pack_wire(qi)


def _kernel_device(x):
    st = _init_device()
    absmax = float(np.abs(x).max())
    if absmax <= INT8_ABSMAX:
        wire = _quant_pack_int8(x)
        scale = 1.0 / INT8_SCALE
    else:
        wire = _pack_wire(x.astype(st["np_bf16"]))
        scale = 1.0
    o = st["fbp"](wire, *st["W"])
    return _assemble(o, scale)


# ---------------- host fallback (exact float32 semantics) ----------------

_HOST = None


def _init_host():
    global _HOST
    if _HOST is not None:
        return _HOST
    F = _filter_matrix()
    KE = _k_eff().reshape(A, N * N)
    k0 = np.clip(np.floor(KE), 0, D - 2).astype(np.int32)
    w = (KE - k0).astype(np.float32)
    gi = (k0 + (np.arange(A, dtype=np.int64) * D)[:, None]).astype(np.int32)
    bp = None
    try:
        import numba

        @numba.njit(fastmath=True, cache=True)
        def bp_(qT, giT, wT, out):
            Pn, nA = giT.shape
            Bc = qT.shape[1]
            acc = np.empty(Bc, np.float32)
            for p in range(Pn):
                r = giT[p, 0]
                w1 = wT[p, 0]
                w0 = np.float32(1.0) - w1
                for ci in range(Bc):
                    acc[ci] = w0 * qT[r, ci] + w1 * qT[r + 1, ci]
                for tt in range(1, nA):
                    r = giT[p, tt]
                    w1 = wT[p, tt]
                    w0 = np.float32(1.0) - w1
                    for ci in range(Bc):
                        acc[ci] += w0 * qT[r, ci] + w1 * qT[r + 1, ci]
                out[p, :] = acc

        bp = bp_
    except Exception:
        bp = None
    _HOST = dict(F=F, k0=k0, w=w, giT=np.ascontiguousarray(gi.T),
                 wT=np.ascontiguousarray(w.T), bp=bp)
    return _HOST


def _kernel_host(x):
    st = _init_host()
    b = x.shape[0]
    q = (x.reshape(b * A, D) @ st["F"]).reshape(b, A * D)
    if st["bp"] is not None:
        qT = np.ascontiguousarray(q.T)
        out = np.empty((N * N, b), np.float32)
        st["bp"](qT, st["giT"], st["wT"], out)
        return np.ascontiguousarray(out.T).reshape(b, N, N)
    out = np.zeros((b, N * N), np.float32)
    q3 = q.reshape(b, A, D)
    for a in range(A):
        qa = q3[:, a, :]
        i0 = st["k0"][a]
        wa = st["w"][a]
        out += (1.0 - wa) * qa[:, i0] + wa * qa[:, i0 + 1]
    return out.reshape(b, N, N)


_DEV_CALL_FAILS = 0


def kernel(x: np.ndarray) -> np.ndarray:
    global _DEV_CALL_FAILS
    x = np.asarray(x, dtype=np.float32)
    if _DEV_CALL_FAILS < 2:  # latch to host after 2 consecutive failed calls
        for _attempt in range(2):  # one retry absorbs transient device resets
            try:
                r = _kernel_device(x)
                _DEV_CALL_FAILS = 0
                return r
            except Exception:
                continue
        _DEV_CALL_FAILS += 1
    return _kernel_host(x)


if __name__ == "__main__":
    rng = np.random.default_rng(0)
    x = rng.standard_normal((B, A, D), dtype=np.float32)
    y = kernel(x)
    print(y.shape, y.dtype, float(np.abs(y).max()))


# revision 43
# speedup vs baseline: 1.0693x; 1.0693x over previous
"""Backward_projection (FBP: ramp filter + backprojection) on 8 trn2 NeuronCores.

Device formulation (hand-written Bass/Tile kernel, one NEFF per core, SPMD):
  * The ramp filter (exact 183x183 circulant-section matmul, all scalar
    factors folded) is FOLDED into the backprojection matrix:
    out[b, p] = sum_d x[b, a, d] * Wt_a[d, p],  Wt_a = F @ tri(d - k[a, p]),
    so the whole FBP is one accumulation of per-angle matmuls on the PE.
  * Angle-flip pairing: k_{284-a}(127-i, j) == k_a(i, j) exactly, so the
    weight tile of (angle a, image row i) also serves (284-a, 127-i); each
    matmul streams N=512 (256 batches x 2 pair-halves) through one
    stationary tile.  Angle 142 (theta = pi/2) pairs with itself: its
    flip-half writes the mirrored row, so it rides the same uniform loop.
  * K = 183 splits into chunks of 128 + 55; the 55-chunks of two adjacent
    pairs are packed into one [110, .] stationary tile (25% fewer streams).
  * Sharding: each core owns the symmetric image-row set
    [8c, 8c+8) u [120-8c, 128-8c) (16 rows, 2048 pixels) for all 256
    batches, accumulated over two 8-row PSUM sweeps.  The sinogram wire is
    PAIR-sharded pair-major (18 pair-slots per core, both flip angles
    adjacent), so the in-kernel AllGather output is consumed directly with
    no DRAM rearrange.
  * Weights (~107MB/core bf16) are GENERATED ON DEVICE once from a small
    geometry table, stored sweep-major, and streamed per call in ~1MB
    chunks (4 pairs per DMA) for near-peak HBM bandwidth.
  * Output is written pixel-major ([16 rows, 128 cols, 256 batches] per
    core) straight from PSUM via DVE copies -- no on-chip transposes; the
    host reassembles (cheap) while fusing the dequant scale.

The wire uses int8 (validated against the reference: quantization noise
contributes ~1.4e-2 of the 2e-2 absmax-relative budget; the bf16 path at
~5e-3 is kept as an automatic fallback for inputs exceeding the int8
range).  Falls back to a host numba/numpy implementation if the device
path is unavailable.
"""

import numpy as np

# --- geometry constants (parallel_beam_geometry on a 128^2 grid) ---
N = 128
CELL = 40.0 / N
RHO = float(np.sqrt(2.0) * 20.0)
A = 285
D = 183
DC = 2.0 * RHO / D
PAD = 512
B = 256
NPAIR = 143          # pairs (t, 284-t) for t<142, plus the self-pair (142,142)
NSLOT = 144          # wire pair-slots (143 + 1 zero pad) = 18 per core
INT8_SCALE = 127.0 / 5.5
INT8_ABSMAX = 5.45  # inputs beyond this use the bf16 wire


def _filter_matrix():
    n = (np.fft.fftfreq(PAD) * PAD).astype(np.int64)
    h = np.zeros(PAD, np.float64)
    h[0] = 1.0 / (4.0 * DC * DC)
    odd = (n % 2) != 0
    h[odd] = -1.0 / (np.pi * n[odd] * DC) ** 2
    idx = (np.arange(D)[None, :] - np.arange(D)[:, None]) % PAD
    return (h[idx] * (12.0 * DC * np.pi / A)).astype(np.float32)


def _k_eff():
    c = -20.0 + (np.arange(N) + 0.5) * CELL
    X, Y = np.meshgrid(c, c, indexing="ij")
    th = (np.arange(A) + 0.5) * np.pi / A
    t = np.cos(th)[:, None] * X.ravel()[None, :] + np.sin(th)[:, None] * Y.ravel()[None, :]
    k = (t - (-RHO + 0.5 * DC)) / DC
    k0 = np.clip(np.floor(k), 0, D - 2)
    w = np.clip(k - k0, 0.0, 1.0)
    return (k0 + w).astype(np.float32).reshape(A, N, N)


_DEV = None
_DEV_FAILED = False


def _init_device():
    global _DEV, _DEV_FAILED
    if _DEV is not None:
        return _DEV
    if _DEV_FAILED:
        raise RuntimeError("device init previously failed")

    import jax
    import jax.numpy as jnp
    import ml_dtypes
    from functools import partial
    import concourse.bass as bass  # noqa: F401  (ensures concourse importable)
    import concourse.mybir as mybir
    from concourse.tile import TileContext
    from concourse.bass2jax import bass_jit, bass_shard_map
    from jax.sharding import Mesh, NamedSharding, PartitionSpec as P
    from jax.experimental.shard_map import shard_map

    BF = mybir.dt.bfloat16
    F32d = mybir.dt.float32
    I8 = mybir.dt.int8

    F = _filter_matrix()
    KE = _k_eff()
    # kg[c, t, i, j]: fractional detector index for pair t at this core's
    # row set (rows [8c,8c+8) u [120-8c,128-8c)), 143 pairs incl. (142,142)
    kg_host = np.stack([
        np.concatenate([KE[0:NPAIR, 8 * c:8 * c + 8, :],
                        KE[0:NPAIR, 120 - 8 * c:128 - 8 * c, :]], axis=1)
        for c in range(8)])                    # [8, 143, 16, 128]

    devs = jax.devices()[:8]
    assert len(devs) == 8
    mesh = Mesh(np.array(devs), ("core",))
    shard0 = NamedSharding(mesh, P("core"))

    NG = (NPAIR + 1) // 2  # 72 hi-det pair-groups (last one half-empty)
    GCH = 3                # AllGather chunks; 18/GCH slots per core per chunk
    SLOC = NSLOT // 8      # 18 pair-slots per core
    CSL = SLOC // GCH      # 6 slots per core per chunk
    # processing order: chunk-major so compute on gathered chunk g overlaps
    # the still-in-flight gather of chunk g+1 (slot 143 is the zero pad and
    # lands last)
    PERM = np.array([18 * c + l for g in range(GCH) for c in range(8)
                     for l in range(CSL * g, CSL * (g + 1))], np.int64)
    NPROC = NSLOT  # 144 processed slots (incl. the zero-padded one)

    @partial(jax.jit, out_shardings=(shard0,) * 2)
    def gen_w(kg, Fd):
        def body(kl, Fd):
            j = jnp.arange(D, dtype=jnp.float32)
            klf = kl.reshape(NPAIR, 16 * 128)
            Wblk = jax.nn.relu(1.0 - jnp.abs(j[None, :, None] - klf[:, None, :]))
            Wt = jnp.einsum("dj,tjq->tdq", Fd, Wblk)        # [143,183,2048]
            # pair (142,142) delivers angle 142 twice to every row (once via
            # flip0, once as the mirror row's flip1) -> halve its weights
            Wt = Wt.at[NPAIR - 1].mul(0.5)
            Wt = Wt.reshape(NPAIR, D, 16, 128)
            # pad the unused slot 143 with zero weights, order by PERM
            Wt = jnp.concatenate(
                [Wt, jnp.zeros((1, D, 16, 128), Wt.dtype)], axis=0)[PERM]
            # DMA-ready sweep-major low-det weights: [2, 36, 128, 4, 8, 128]
            # (chunk T = processing slots 4T..4T+3)
            w0p = Wt[:, 0:128].reshape(36, 4, 128, 16, 128).transpose(0, 2, 1, 3, 4)
            w0 = jnp.stack([w0p[..., 0:8, :], w0p[..., 8:16, :]],
                           axis=0).astype(jnp.bfloat16)
            # packed hi-det weights: [2, 36, 110, 2, 8, 128] (group g =
            # processing slots 2g, 2g+1; chunk = 2 groups)
            w1s = Wt[:, 128:183, :, :]                       # [144, 55, 16, 128]
            w1s = w1s.reshape(36, 2, 110, 16, 128).transpose(0, 2, 1, 3, 4)
            w1 = jnp.stack([w1s[..., 0:8, :], w1s[..., 8:16, :]],
                           axis=0).astype(jnp.bfloat16)
            return w0, w1
        return shard_map(body, mesh=mesh, in_specs=(P("core"), P()),
                         out_specs=(P("core"),) * 2, check_rep=False)(kg, Fd)

    def make_fbp(R, ag="shared", nbufs=3, ribufs=6, dpbufs=2, diag=None,
                 trace_sim=False):
        # R = number of back-to-back repetitions of the full per-call
        # pipeline inside one NEFF.  R=1 is the production kernel; R>1
        # variants exist only so the marginal per-repetition device time can
        # be measured (the axon relay adds ~70ms of fixed per-dispatch RPC
        # overhead that would otherwise swamp the sub-ms kernel).
        # ag: "shared"|"internal" AllGather buffer kind, "off" = timing
        # ablation only (wrong data).

        @bass_jit(num_devices=8)
        def fbp_bass(nc, xs, w0, w1):
            # xs [18, 183, 2, 256]: this core's pair-slots, det-major
            # w0 [2, 36, 128, 4, 8, 128], w1 [2, 36, 110, 2, 8, 128]
            wire_dt = I8 if xs.dtype == I8 else BF
            out = nc.dram_tensor("out", [128, 16, 256], BF, kind="ExternalOutput")
            with TileContext(nc, trace_sim=trace_sim) as tc:
                with tc.tile_pool(name="wp", bufs=nbufs) as wp, \
                     tc.tile_pool(name="rp", bufs=ribufs) as rp, \
                     tc.tile_pool(name="ri", bufs=ribufs) as ri, \
                     tc.tile_pool(name="ps", bufs=8, space="PSUM") as ps, \
                     tc.tile_pool(name="sp", bufs=dpbufs) as sp, \
                     tc.tile_pool(name="dp", bufs=dpbufs, space="DRAM") as dp:
                 for _rep in range(R):
                    bounce_in = dp.tile([SLOC, D, 2, 256], wire_dt, tag="bin")
                    nc.sync.dma_start(out=bounce_in, in_=xs[:, :, :, :])
                    if ag == "one":
                        # single collective: lowest entry/exit overhead; the
                        # dpbufs=2 double buffering hides it under the
                        # previous repetition's compute in steady state
                        bout1 = dp.tile([8, SLOC, D, 2, 256], wire_dt,
                                        tag="bout1", name="bout1",
                                        addr_space="Shared")
                        nc.gpsimd.collective_compute(
                            "AllGather", mybir.AluOpType.bypass,
                            replica_groups=[list(range(8))],
                            ins=[bounce_in], outs=[bout1])
                        bounce_out = [bout1]
                    else:
                        bounce_out = [
                            dp.tile([8, CSL, D, 2, 256], wire_dt, tag=f"bout{g}",
                                    name=f"bout{g}",
                                    addr_space="Shared" if ag == "shared" else "Local")
                            for g in range(GCH)]
                        if ag != "off":
                            for g in range(GCH):
                                nc.gpsimd.collective_compute(
                                    "AllGather", mybir.AluOpType.bypass,
                                    replica_groups=[list(range(8))],
                                    ins=[bounce_in[CSL * g:CSL * (g + 1)]],
                                    outs=[bounce_out[g]])

                    def slot(t):  # processing index -> location in bounce_out
                        s = int(PERM[t])
                        c, l = s // SLOC, s % SLOC
                        return (l // CSL, c, l % CSL)

                    def bsrc(gc, c, l):  # bounce slice for (chunk, core, local)
                        if ag == "one":
                            return bounce_out[0][c, gc * CSL + l]
                        return bounce_out[gc][c, l]

                    stg = sp.tile([128, 16, 256], BF, tag="stg")

                    # input loads ride the ACT HWDGE ring so they never queue
                    # FIFO behind the 1MB weight chunks on the SP ring
                    in_dma = nc.sync if diag == "syncin" else nc.scalar
                    phased = diag in ("phased", "ringph")
                    if diag == "rhsfix":  # timing ablation: resident rhs
                        r0fix = sp.tile([128, 2, 256], BF, tag="r0fix",
                                        name=f"r0fix_{_rep}")
                        nc.vector.memset(r0fix, 0.25)
                        r1fix = sp.tile([110, 2, 256], BF, tag="r1fix",
                                        name=f"r1fix_{_rep}")
                        nc.vector.memset(r1fix, 0.25)

                    def do_w1(sw, g, accs, last):
                        nonlocal w1c
                        if g % 2 == 0:
                            w1c = wp.tile([110, 2, 8, 128], BF, tag="w1",
                                          name=f"w1_{sw}_{g}")
                            nc.sync.dma_start(out=w1c, in_=w1[sw, g // 2])
                        if diag == "rhsfix":
                            r1 = r1fix
                        else:
                            r1i = ri.tile([110, 2, 256], wire_dt, tag="r1i",
                                          name=f"r1i_{sw}_{g}")
                            for kk in range(2):
                                gca, ca, la = slot(2 * g + kk)
                                in_dma.dma_start(
                                    out=r1i[55 * kk:55 * kk + 55],
                                    in_=bsrc(gca, ca, la)[128:183])
                            if wire_dt == BF:
                                r1 = r1i
                            else:
                                r1 = rp.tile([110, 2, 256], BF, tag="r1",
                                             name=f"r1_{sw}_{g}")
                                nc.vector.tensor_copy(r1, r1i)
                        for r in range(8):
                            nc.tensor.matmul(accs[r], lhsT=w1c[:, g % 2, r, :],
                                             rhs=r1, start=False, stop=last)

                    for sw in range(2):
                        accs = [ps.tile([128, 512], F32d, tag="acc",
                                        name=f"acc{sw}_{r}") for r in range(8)]
                        w0c = w1c = None
                        for t in range(NPROC):
                            if t % 4 == 0:
                                w0c = wp.tile([128, 4, 8, 128], BF, tag="w0",
                                              name=f"w0_{sw}_{t}")
                                nc.sync.dma_start(out=w0c, in_=w0[sw, t // 4])
                                if diag == "wx2":
                                    nc.sync.dma_start(out=w0c, in_=w0[sw, t // 4])
                            if diag == "rhsfix":
                                r0 = r0fix
                            else:
                                gc, c, l = slot(t)
                                r0i = ri.tile([128, 2, 256], wire_dt, tag="r0i",
                                              name=f"r0i_{sw}_{t}")
                                in_dma.dma_start(out=r0i,
                                                 in_=bsrc(gc, c, l)[0:128])
                                if wire_dt == BF:
                                    r0 = r0i
                                else:
                                    r0 = rp.tile([128, 2, 256], BF, tag="r0",
                                                 name=f"r0_{sw}_{t}")
                                    nc.vector.tensor_copy(r0, r0i)
                                    if diag == "dve2":
                                        nc.vector.tensor_copy(r0, r0i)
                            for r in range(8):
                                nc.tensor.matmul(accs[r], lhsT=w0c[:, t % 4, r, :],
                                                 rhs=r0, start=(t == 0), stop=False)
                            if not phased and t % 2 == 1:
                                do_w1(sw, t // 2, accs, last=(t == NPROC - 1))
                        if phased:
                            for g in range(NPROC // 2):
                                do_w1(sw, g, accs, last=(g == NPROC // 2 - 1))

                        for r in range(8):
                            # flip0 -> slot (8*sw + r); flip1 -> mirrored slot
                            s0 = 8 * sw + r
                            s1 = (15 - r) if sw == 0 else (7 - r)
                            if sw == 0:
                                nc.vector.tensor_copy(stg[:, s0, :], accs[r][:, 0:256])
                                nc.vector.tensor_copy(stg[:, s1, :], accs[r][:, 256:512])
                            else:
                                nc.vector.tensor_add(stg[:, s0, :], stg[:, s0, :],
                                                     accs[r][:, 0:256])
                                nc.vector.tensor_add(stg[:, s1, :], stg[:, s1, :],
                                                     accs[r][:, 256:512])
                    nc.sync.dma_start(out=out[:, :, :], in_=stg[:, :, :])
            return out

        return bass_shard_map(fbp_bass, mesh=mesh,
                              in_specs=(P("core"),) * 3, out_specs=P("core"))

    fbp_dev = make_fbp(1)

    try:
        W0g, W1g = gen_w(kg_host.reshape(8 * NPAIR, 16, 128), F)
        for t_ in (W0g, W1g):
            t_.block_until_ready()
        # compile the int8-wire NEFF now so the first real call is fast
        dummy = np.zeros((NSLOT, D, 2, 256), np.int8)
        o0 = fbp_dev(dummy, W0g, W1g)
        o0.block_until_ready()
    except Exception:
        _DEV_FAILED = True
        raise

    _DEV = dict(jax=jax, np_bf16=np.dtype(ml_dtypes.bfloat16),
                fbp=fbp_dev, W=(W0g, W1g), in_sharding=shard0,
                make_fbp=make_fbp)
    return _DEV


_ROWSETS = [np.r_[8 * c:8 * c + 8, 120 - 8 * c:128 - 8 * c] for c in range(8)]


def _assemble(on, scale):
    # on: [1024, 16, 256] bf16-ish from device = [8 cores x 128 cols,
    # 16 row-slots, 256 batches]; reassemble to [256, 128, 128] f32 with
    # the dequant scale fused into the cast
    on = np.asarray(on).reshape(8, N, 16, B)
    img = np.empty((B, N, N), np.float32)
    for c in range(8):
        blk = np.multiply(on[c].transpose(2, 1, 0), scale, dtype=np.float32)
        img[:, _ROWSETS[c], :] = blk
    return img


_PACK_BUF = None


def _pack_wire(xq):
    # [256,285,183] -> pair-major pair-sharded wire [144, 183, 2, 256]:
    # slot t < 142: (angle t, angle 284-t); slot 142: (142, 142); slot 143: 0
    global _PACK_BUF
    if _PACK_BUF is None or _PACK_BUF.dtype != xq.dtype:
        _PACK_BUF = np.zeros((NSLOT, D, 2, B), xq.dtype)
    w = _PACK_BUF
    # xq[b, a, d] -> w[t, d, 0, b] = xq[b, t, d]; w[t, d, 1, b] = xq[b, 284-t, d]
    xt = xq.transpose(1, 2, 0)                       # [285, 183, 256]
    w[0:NPAIR, :, 0, :] = xt[0:NPAIR]
    w[0:142, :, 1, :] = xt[284:142:-1]
    w[142, :, 1, :] = xt[142]
    return w


_QBUFS = None
_QUANT_NUMBA = None


def _get_quant_numba():
    # single-pass fused quantizer: int8(rint(x*s)); ~2x the numpy 3-pass chain
    global _QUANT_NUMBA
    if _QUANT_NUMBA is None:
        try:
            import numba

            @numba.njit(cache=True)
            def q_(xf, s, out):
                for i in range(xf.size):
                    out[i] = np.int8(np.rint(xf[i] * s))

            q_(np.zeros(4, np.float32), 1.0, np.empty(4, np.int8))  # warm JIT
            _QUANT_NUMBA = q_
        except Exception:
            _QUANT_NUMBA = False
    return _QUANT_NUMBA


def _quant_pack_int8(x):
    global _QBUFS
    if _QBUFS is None:
        _QBUFS = (np.empty(x.shape, np.float32), np.empty(x.shape, np.int8))
    tmp, qi = _QBUFS
    qn = _get_quant_numba()
    if qn:
        qn(x.reshape(-1), INT8_SCALE, qi.reshape(-1))
    else:
        np.multiply(x, INT8_SCALE, out=tmp)
        np.rint(tmp, out=tmp)
        np.copyto(qi, tmp, casting="unsafe")
    return _pack_wire(qi)


def _kernel_device(x):
    st = _init_device()
    absmax = float(np.abs(x).max())
    if absmax <= INT8_ABSMAX:
        wire = _quant_pack_int8(x)
        scale = 1.0 / INT8_SCALE
    else:
        wire = _pack_wire(x.astype(st["np_bf16"]))
        scale = 1.0
    o = st["fbp"](wire, *st["W"])
    return _assemble(o, scale)


# ---------------- host fallback (exact float32 semantics) ----------------

_HOST = None


def _init_host():
    global _HOST
    if _HOST is not None:
        return _HOST
    F = _filter_matrix()
    KE = _k_eff().reshape(A, N * N)
    k0 = np.clip(np.floor(KE), 0, D - 2).astype(np.int32)
    w = (KE - k0).astype(np.float32)
    gi = (k0 + (np.arange(A, dtype=np.int64) * D)[:, None]).astype(np.int32)
    bp = None
    try:
        import numba

        @numba.njit(fastmath=True, cache=True)
        def bp_(qT, giT, wT, out):
            Pn, nA = giT.shape
            Bc = qT.shape[1]
            acc = np.empty(Bc, np.float32)
            for p in range(Pn):
                r = giT[p, 0]
                w1 = wT[p, 0]
                w0 = np.float32(1.0) - w1
                for ci in range(Bc):
                    acc[ci] = w0 * qT[r, ci] + w1 * qT[r + 1, ci]
                for tt in range(1, nA):
                    r = giT[p, tt]
                    w1 = wT[p, tt]
                    w0 = np.float32(1.0) - w1
                    for ci in range(Bc):
                        acc[ci] += w0 * qT[r, ci] + w1 * qT[r + 1, ci]
                out[p, :] = acc

        bp = bp_
    except Exception:
        bp = None
    _HOST = dict(F=F, k0=k0, w=w, giT=np.ascontiguousarray(gi.T),
                 wT=np.ascontiguousarray(w.T), bp=bp)
    return _HOST


def _kernel_host(x):
    st = _init_host()
    b = x.shape[0]
    q = (x.reshape(b * A, D) @ st["F"]).reshape(b, A * D)
    if st["bp"] is not None:
        qT = np.ascontiguousarray(q.T)
        out = np.empty((N * N, b), np.float32)
        st["bp"](qT, st["giT"], st["wT"], out)
        return np.ascontiguousarray(out.T).reshape(b, N, N)
    out = np.zeros((b, N * N), np.float32)
    q3 = q.reshape(b, A, D)
    for a in range(A):
        qa = q3[:, a, :]
        i0 = st["k0"][a]
        wa = st["w"][a]
        out += (1.0 - wa) * qa[:, i0] + wa * qa[:, i0 + 1]
    return out.reshape(b, N, N)


_DEV_CALL_FAILS = 0


def kernel(x: np.ndarray) -> np.ndarray:
    global _DEV_CALL_FAILS
    x = np.asarray(x, dtype=np.float32)
    if _DEV_CALL_FAILS < 2:  # latch to host after 2 consecutive failed calls
        for _attempt in range(2):  # one retry absorbs transient device resets
            try:
                r = _kernel_device(x)
                _DEV_CALL_FAILS = 0
                return r
            except Exception:
                continue
        _DEV_CALL_FAILS += 1
    return _kernel_host(x)


if __name__ == "__main__":
    rng = np.random.default_rng(0)
    x = rng.standard_normal((B, A, D), dtype=np.float32)
    y = kernel(x)
    print(y.shape, y.dtype, float(np.abs(y).max()))
